# revision 54
# baseline (speedup 1.0000x reference)
"""Stereo cost-volume + softmax disparity regression + bilinear upsample.

Full inputs:  feat_l, feat_r [16, 4, 128, 240] f32, img_h=1024, img_w=1920.
Full output:  [16, 1, 1024, 1920] f32.

Sharding: pure data parallel, 2 samples per core across 8 cores; the two
samples run as a software pipeline (sample 1's cost volume overlaps
sample 0's upsample).

Phase 1 (7 disparity groups per sample: 5x4 + 2x2, the small ones last so
the final [abs -> cs -> exp -> st -> pred] chain is short):
  - Subtract |L - R(x-d)| for a whole group in ONE tensor_tensor (custom
    4D access pattern walking the host-pre-padded feat_r window at
    stride 1); groups split between Pool and DVE per CFG.  The u16
    bitwise abs runs on DVE (4x mode, ~1ns/4el).
  - Channel sum runs "flipped" on the PE: the diff chunk [128, 120] is
    the stationary lhsT and the [128, 32] selector streams, producing
    cost chunks [120(x), 32(y32)].  Layout: cost[x, (sec, yb, xb, y32)].
  - ACT exponentiates a whole group tile [120, <=1024] at once.
  - s/t accumulate in PSUM via TWO fat matmuls per section (s += e,
    t += 8d*e over a whole [120, 256] section; st layout
    [s(yb,xb,y32) | t(yb,xb,y32)]); PSUM lazy-zero semantics allow
    interleaved region accumulation with start exactly once per bank.
Phase 2: pred = t * (1/s) comes out ALREADY x-transposed
  [120(x), (yb, xb, y32)], so M1 (x-interp) consumes per-xb strided
  [120, 4, 32] slices as lhsT -- no PE transposes.  M1 writes one merged
  tmp [128, 1920]; M2 (y-interp) reads arbitrary 512-col slices.
  PSUM->SBUF copies go to ACT and DVE per CFG lane patterns (GPSIMD
  cannot read PSUM).  Mid rows (overlapped with s1's phase 1) use
  1-bank PSUM chunks; the tail uses 4-bank tiles with one fat full-row
  copy per row and half-row output DMAs on a sweepable queue pattern.

Constants are packed into two DMA blobs (sel+sid+tid, wyT+wxTa+wxTb) to
cut descriptor floors.  All engine assignments live in CFG, tuned by
sweeping CoreSim.
"""
import sys

sys.path.insert(0, "/opt/trn_rl_repo")

import numpy as np

import concourse.bass as bass
import concourse.bacc as bacc
import concourse.tile as tile
import concourse.mybir as mybir
from concourse.bass_utils import run_bass_kernel_spmd

# ---------------------------------------------------------------- constants
B, C, H0, W0 = 16, 4, 128, 240
D = 24             # disparities
NCORES = 8
SPC = B // NCORES  # samples per core = 2
HI, WI = 1024, 1920
WP = WI
XB = 120           # x-block width (two blocks per row)
# M2 / output X chunks (PSUM <= 512 cols each)
XCH = [(0, 512), (512, 512), (1024, 512), (1536, 384)]
# M1 X chunks: (start, width, x-halves needed); 956/964 is the exact
# pure-A/pure-B wxT boundary (only an 8-column sliver needs both halves)
XCH_M1 = [(0, 512, (0,)), (512, 444, (0,)), (956, 8, (0, 1)),
          (964, 504, (1,)), (1468, 452, (1,))]
YB = H0 // 32      # 4 y-blocks
G = SPC * YB       # 8 feat groups (sample-major)
FREE = G * W0      # 1920
PAD = 28           # left-pad columns in padded feat_r groups
GW = W0 + 2 * PAD  # padded group width (even)
EXP_BIAS = 8.0

# consts blob1 layout: sel [128,32] | sid [120,120]; tid loads separately
B1_SEL, B1_SID = 0, 32
B1_W = 152
# consts blob2 layout: wyT [128,1024] | wxTa [120,1920] | wxTb [120,1920]
B2_WY, B2_WXA, B2_WXB = 0, HI, HI + WI
B2_W = HI + 2 * WI

FP16 = mybir.dt.float16
F32 = mybir.dt.float32
U16 = mybir.dt.uint16

_TRACE = [False]


# ------------------------------------------------------------- host weights
def _host_consts():
    # selector for the flipped channel sum: sel[ch*32+y32, y'] = (y32 == y')
    sel = np.zeros((128, 32), np.float16)
    for ch in range(C):
        sel[ch * 32 : (ch + 1) * 32, :] = np.eye(32, dtype=np.float16)

    # s identity and per-disparity t identities (8*d scaling)
    sid = np.eye(XB, dtype=np.float16)
    tid = np.zeros((XB, D * XB), np.float16)
    for d in range(D):
        tid[:, d * XB : (d + 1) * XB] = np.eye(XB, dtype=np.float16) * \
            np.float16(8.0 * d)

    # x-interp weights wxT[x, X], f32 linspace to match jnp rounding
    xs = np.linspace(0.0, W0 - 1.0, WI, dtype=np.float32)
    x0 = np.floor(xs).astype(np.int64)
    x1 = np.minimum(x0 + 1, W0 - 1)
    wx = (xs - x0).astype(np.float32)
    wxT_full = np.zeros((W0, WI), np.float32)
    wxT_full[x0, np.arange(WI)] += 1.0 - wx
    wxT_full[x1, np.arange(WI)] += wx
    # chunk validity: columns left of 956 only use x<120; right of 964 only
    # x>=120; the 8-col sliver uses both
    assert x1[:956].max() <= XB - 1
    assert x0[964:].min() >= XB
    wxTa = wxT_full[0:XB]
    wxTb = wxT_full[XB : 2 * XB]

    # y-interp weights wyT[y, Y]
    ys = np.linspace(0.0, H0 - 1.0, HI, dtype=np.float32)
    y0 = np.floor(ys).astype(np.int64)
    y1 = np.minimum(y0 + 1, H0 - 1)
    wy = (ys - y0).astype(np.float32)
    wyT = np.zeros((H0, HI), np.float32)
    wyT[y0, np.arange(HI)] += 1.0 - wy
    wyT[y1, np.arange(HI)] += wy

    cst1 = np.zeros((128, B1_W), np.float16)
    cst1[:, B1_SEL:B1_SID] = sel
    cst1[0:XB, B1_SID:B1_W] = sid
    cst2 = np.zeros((128, B2_W), np.float16)
    cst2[:, B2_WY:B2_WXA] = wyT.astype(np.float16)
    cst2[0:XB, B2_WXA:B2_WXB] = wxTa.astype(np.float16)
    cst2[0:XB, B2_WXB:B2_W] = wxTb.astype(np.float16)
    return {"cst1": cst1, "cst2": cst2, "ctid": tid}


def _pack_feat(f):
    """[SPC, C, H0, W0] -> [128, FREE] with p=(ch,y32), free=(s,yb,x)."""
    a = f.reshape(SPC, C, YB, 32, W0)
    a = np.ascontiguousarray(a.transpose(1, 3, 0, 2, 4))  # ch,y32,s,yb,x
    return a.reshape(128, FREE)


def _pack_feat_padded(f):
    """[SPC, C, H0, W0] -> [128, SPC*YB*GW], PAD zero cols around each row."""
    a = f.reshape(SPC, C, YB, 32, W0).transpose(1, 3, 0, 2, 4)
    p = np.zeros((C, 32, SPC, YB, GW), f.dtype)
    p[:, :, :, :, PAD : PAD + W0] = a
    return p.reshape(128, SPC * YB * GW)


# scheduling configuration (engine assignment knobs, tuned via sweep).
# orderN: per-sample phase-1 emission order of the 12 two-disparity
# groups, each tagged with its subtract engine (P=Pool, V=DVE).
_O = "VPVPPVPPVPPP"
CFG = {
    "order0": tuple((i, _O[i]) for i in range(12)),
    "order1": tuple((i, _O[i]) for i in range(12)),
    "mid_pat": "AV",           # PSUM->SBUF copy lanes, mid rows
    "tail_pat": "AV",          # tail rows
    "mid_dma": "S",            # output DMA queues, mid rows (full row)
    "tail_dma": "SP",          # tail half-row DMA queue pattern
    "mid_rows": 6,             # s0 rows emitted during s1 phase 1
    "lf0_pool": False,         # load lf sample-0 half on Pool's queue
    "lastrow": 2,              # trailing rows with 4-way chunked DMA
    "divide": False,            # pred via DVE divide (vs recip+mult)
    "fat_mid": False,          # mid rows: one 4-bank tile + one fat copy
    "fat_tail": False,         # tail rows: 4-bank tiles + one fat copy
    "tail_chunks": 2,          # tail row chunking: 2x1024 or 4x512
    "lr_q": "SPPS",            # queues for the 4 chunked last-row DMAs
}


# ------------------------------------------------------------- build kernel
def _build(cfg=None):
    cfg = {**CFG, **(cfg or {})}
    nc = bacc.Bacc("TRN2", target_bir_lowering=False, debug=False,
                   num_devices=NCORES)
    lf = nc.dram_tensor("lf", [128, FREE], FP16, kind="ExternalInput").ap()
    rf = nc.dram_tensor("rf", [128, SPC * YB * GW], FP16,
                        kind="ExternalInput").ap()
    cst1_d = nc.dram_tensor("cst1", [128, B1_W], FP16,
                            kind="ExternalInput").ap()
    ctid_d = nc.dram_tensor("ctid", [XB, D * XB], FP16,
                            kind="ExternalInput").ap()
    cst2_d = nc.dram_tensor("cst2", [128, B2_W], FP16,
                            kind="ExternalInput").ap()
    out = nc.dram_tensor("out", [SPC, HI, WI], FP16,
                         kind="ExternalOutput").ap()

    AF = mybir.ActivationFunctionType
    OP = mybir.AluOpType

    with tile.TileContext(nc) as tc:
        with (
            tc.tile_pool(name="consts", bufs=1) as consts,
            tc.tile_pool(name="feat", bufs=1) as feat,
            tc.tile_pool(name="diff", bufs=8) as diffp,
            tc.tile_pool(name="ep", bufs=6) as ep,
            tc.tile_pool(name="predp", bufs=1) as predp,
            tc.tile_pool(name="upsb", bufs=1) as upsb,
            tc.tile_pool(name="outsb", bufs=6) as outsb,
        ):
            from contextlib import ExitStack
            ph1_stack = ExitStack()
            # PSUM budget during phase 1 + mid: cost 2x1 bank (double-
            # buffered 2-disp groups) + st0/st1 (2) + two 2-bank output
            # tiles (4) = 8.
            costp = ph1_stack.enter_context(
                tc.tile_pool(name="costp", bufs=2, space="PSUM"))
            stps = ph1_stack.enter_context(
                tc.tile_pool(name="stps", bufs=1, space="PSUM"))
            outps = ph1_stack.enter_context(
                tc.tile_pool(name="outps",
                             bufs=1 if cfg["fat_mid"] else 2,
                             space="PSUM"))

            bias8 = consts.tile([XB, 1], F32)
            nc.vector.memset(bias8, EXP_BIAS)

            # ---- input DMAs spread across the four queues so everything
            # lands early: lf halves + blob2 on SP, rf h0 on Pool (feeds
            # Pool's first subtract), rf h1 on DVE (idle at start), the
            # phase-1 consts blob on ACT.
            # SP queue order: lf-s0, cb1 (needed ~5us), rf-s1, lf-s1, cb2
            # (needed at M1 ~15us).  rf-s0 rides Pool's own queue so its
            # first subtract follows immediately.
            Lh, R = [], [None, None]
            Rt0 = feat.tile([128, YB * GW], FP16, tag="rpad0", name="rpad0")
            nc.gpsimd.dma_start(out=Rt0, in_=rf[:, 0 : YB * GW])
            Lt0 = feat.tile([128, FREE // 2], FP16, tag="L0", name="L0")
            lf0_eng = nc.gpsimd if cfg["lf0_pool"] else nc.sync
            lf0_eng.dma_start(out=Lt0, in_=lf[:, 0 : FREE // 2])
            cb1 = consts.tile([128, B1_W], FP16, name="cb1", tag="cb1")
            nc.scalar.dma_start(out=cb1, in_=cst1_d)
            tid = consts.tile([XB, D * XB], FP16, name="tid", tag="tid")
            nc.sync.dma_start(out=tid, in_=ctid_d)
            Rt1 = feat.tile([128, YB * GW], FP16, tag="rpad1", name="rpad1")
            nc.sync.dma_start(out=Rt1, in_=rf[:, YB * GW : 2 * YB * GW])
            Lt1 = feat.tile([128, FREE // 2], FP16, tag="L1", name="L1")
            nc.sync.dma_start(out=Lt1, in_=lf[:, FREE // 2 : FREE])
            cb2 = consts.tile([128, B2_W], FP16, name="cb2", tag="cb2")
            nc.sync.dma_start(out=cb2, in_=cst2_d)
            for Lt in (Lt0, Lt1):
                Lh.append(Lt.rearrange("p (g w) -> p g w", w=W0))
            for h2, Rt in enumerate((Rt0, Rt1)):
                R[h2] = Rt.rearrange("p (g w) -> p g w", w=GW)

            sel = cb1[:, B1_SEL:B1_SID]
            sid = cb1[0:XB, B1_SID:B1_W]
            wyT = cb2[:, B2_WY:B2_WXA]
            wxT = [cb2[0:XB, B2_WXA:B2_WXB], cb2[0:XB, B2_WXB:B2_W]]

            st = [stps.tile([XB, 512], F32, name=f"st{h}", tag=f"st{h}")
                  for h in range(SPC)]

            # ---------- copy lanes
            mid_tick = [0]
            tail_tick = [0]

            def _copy_on(eng, dst, src):
                # PSUM->SBUF moves: ACT or DVE only (GPSIMD cannot touch
                # PSUM -- the BIR verifier rejects it)
                if eng == "A":
                    nc.scalar.copy(out=dst, in_=src)
                else:
                    nc.vector.tensor_copy(out=dst, in_=src)

            MID_PAT = list(cfg["mid_pat"])
            TAIL_PAT = list(cfg["tail_pat"])
            MID_DMA = list(cfg["mid_dma"])
            TAIL_DMA = list(cfg["tail_dma"])
            ENG = {"S": nc.sync, "A": nc.scalar, "P": nc.gpsimd,
                   "V": nc.vector}

            def copy_mid(dst, src):
                _copy_on(MID_PAT[mid_tick[0] % len(MID_PAT)], dst, src)
                mid_tick[0] += 1

            def copy_tail(dst, src):
                _copy_on(TAIL_PAT[tail_tick[0] % len(TAIL_PAT)], dst, src)
                tail_tick[0] += 1

            # ============ software pipeline over the two samples =========
            pred = [None, None]
            # diff tile sections hold disparities hi-first: [d+3,d+2,d+1,d]
            st_open = [False, False]

            def emit_ph1_group(h, d0, nsec, eng=None, last_g=False):
                eng = eng or nc.vector
                Dt = diffp.tile([128, 2 * YB * W0], FP16, name="diff",
                                tag="diff")
                D4 = Dt.rearrange("p (s g w) -> p s g w", s=2, w=W0)[:, 0:nsec]
                Lk = Lh[h].unsqueeze(1).broadcast_to([128, nsec, YB, W0])
                # one subtract covers disparities d0+nsec-1..d0 via a k-dim
                # stepping the feat_r window right by 1
                off_hi = PAD - (d0 + nsec - 1)
                Rbase = R[h][:, :, off_hi : off_hi + W0]
                Rk = bass.AP(
                    Rbase.tensor, Rbase.offset,
                    [list(Rbase.ap[0]), [1, nsec],
                     list(Rbase.ap[1]), list(Rbase.ap[2])])
                eng.tensor_tensor(out=D4, in0=Lk, in1=Rk, op=OP.subtract)
                Du = Dt.bitcast(U16)[:, 0 : nsec * YB * W0]
                nc.vector.tensor_scalar(
                    out=Du, in0=Du, scalar1=0x7FFF, scalar2=None,
                    op0=OP.bitwise_and,
                )
                D3 = Dt.rearrange("p (s f) -> p s f", s=2)
                # flipped channel sum: cost[x, (sec, xb, yb, y32)] --
                # xb-major so pred's per-xb M1 weight slices are contiguous
                cost = costp.tile([XB, 512], F32, name="cost", tag="cost")
                e = ep.tile([XB, 512], FP16, name="e", tag="e")
                for sec in range(nsec):
                    for yb in range(YB):
                        for xb in range(2):
                            nc.tensor.matmul(
                                cost[0:XB,
                                     sec * 256 + xb * 128 + yb * 32 :
                                     sec * 256 + xb * 128 + yb * 32 + 32],
                                lhsT=D3[:, sec,
                                        yb * W0 + xb * XB :
                                        yb * W0 + xb * XB + XB],
                                rhs=sel,
                                start=(sec == 0 and yb == 0 and xb == 0),
                                stop=(sec == nsec - 1
                                      and yb == YB - 1 and xb == 1),
                                skip_group_check=True,
                            )
                nc.scalar.activation(out=e[:, 0 : nsec * 256],
                                     in_=cost[:, 0 : nsec * 256],
                                     func=AF.Exp, bias=bias8, scale=-1.0)
                for sec in range(nsec):
                    _emit_st_sec(h, d0 + (nsec - 1 - sec), sec, e, last_g
                                 and sec == nsec - 1)

            def _emit_st_sec(h, d, sec, e, last_sec):
                # fat accumulate over a whole [120, 256] section:
                # st layout [s(yb,xb,y32) | t(yb,xb,y32)] matches e's order
                first = not st_open[h]
                st_open[h] = True
                rhs = e[:, sec * 256 : sec * 256 + 256]
                nc.tensor.matmul(
                    st[h][0:XB, 0:256], lhsT=sid, rhs=rhs,
                    start=first, stop=False, skip_group_check=True,
                )
                nc.tensor.matmul(
                    st[h][0:XB, 256:512],
                    lhsT=tid[:, d * XB : d * XB + XB], rhs=rhs,
                    start=False, stop=last_sec, skip_group_check=True,
                )

            def emit_pred(h):
                pr = predp.tile([XB, 256], FP16, name=f"pred{h}",
                                tag=f"pred{h}")
                if cfg["divide"]:
                    nc.vector.tensor_tensor(out=pr, in0=st[h][:, 256:512],
                                            in1=st[h][:, 0:256],
                                            op=OP.divide)
                else:
                    rs = predp.tile([XB, 256], F32, name=f"rs{h}",
                                    tag=f"rs{h}")
                    nc.vector.reciprocal(out=rs, in_=st[h][:, 0:256])
                    nc.vector.tensor_tensor(out=pr, in0=st[h][:, 256:512],
                                            in1=rs, op=OP.mult)
                # pred layout [x, (xb, yb, y32)] = [x, (xb, y128)]
                pred[h] = pr

            def mid_tile():
                if cfg["fat_mid"]:
                    return outps.tile([128, 2048], F32, name="o_ps",
                                      tag="o_ps")
                return outps.tile([128, 1024], F32, name="o_ps",
                                  tag="o_ps")

            def emit_ph2_head(h, copy_fn, pool_fns, narrow=False):
                """M1 -> merged tmp [128, WP].  narrow: per-chunk tiles
                (<=512 wide); else two fat half copies from >=2-bank
                tiles."""
                lhs = [pred[h][:, xb * 128 : xb * 128 + 128]
                       for xb in range(2)]
                tmp = upsb.tile([128, WP], FP16, tag=f"tmp{h}",
                                name=f"tmp{h}")
                if narrow:
                    for c0, nw, halves in XCH_M1:
                        t_ps = pool_fns[0]()
                        for i, xb in enumerate(halves):
                            nc.tensor.matmul(
                                t_ps[:, 0:nw], lhsT=lhs[xb],
                                rhs=wxT[xb][:, c0 : c0 + nw],
                                start=(i == 0), stop=(i == len(halves) - 1),
                            )
                        copy_fn(tmp[:, c0 : c0 + nw], t_ps[:, 0:nw])
                    return tmp
                for half, (lo, hi, base, wtot) in enumerate(
                        ((0, 2, 0, 956), (2, 5, 956, WP - 956))):
                    t_ps = pool_fns[half]()
                    for c0, nw, halves in XCH_M1[lo:hi]:
                        for i, xb in enumerate(halves):
                            nc.tensor.matmul(
                                t_ps[:, c0 - base : c0 - base + nw],
                                lhsT=lhs[xb],
                                rhs=wxT[xb][:, c0 : c0 + nw],
                                start=(i == 0),
                                stop=(i == len(halves) - 1),
                            )
                    copy_fn(tmp[:, base : base + wtot], t_ps[:, 0:wtot])
                return tmp

            mid_dma_tick = [0]
            tail_dma_tick = [0]

            def emit_ph2_yc(h, tmp, yc, copy_fn, tail_tile=None,
                            last_row=False):
                """tail_tile=None: two 2-bank chunks + one full-row DMA
                (mid rows).  tail_tile: 2-bank tiles, one copy + one DMA
                per half-row (tail).  last_row: 4-way chunked DMA across
                queues to shorten the final drain."""
                ob = outsb.tile([128, WP], FP16, name="ob", tag="ob")
                wy_col = wyT[:, yc * 128 : yc * 128 + 128]
                tile_fn = tail_tile or mid_tile
                fat = cfg["fat_tail"] if tail_tile else cfg["fat_mid"]
                if tail_tile is not None and cfg["tail_chunks"] == 4:
                    chunks = tuple(XCH)
                elif fat:
                    chunks = ((0, WP),)
                else:
                    chunks = ((0, 1024), (1024, WP - 1024))
                for c0, cw in chunks:
                    o_ps = tile_fn()
                    for xc0, xnw in XCH:
                        if xc0 < c0 or xc0 >= c0 + cw:
                            continue
                        nc.tensor.matmul(
                            o_ps[:, xc0 - c0 : xc0 - c0 + xnw],
                            lhsT=wy_col, rhs=tmp[:, xc0 : xc0 + xnw],
                            start=True, stop=True,
                        )
                    copy_fn(ob[:, c0 : c0 + cw], o_ps[:, 0:cw])
                    if tail_tile is None:
                        continue
                    if last_row:
                        hw_ = cw // 2
                        lq = cfg["lr_q"]
                        for q, (d0, dw) in zip(
                                lq[0:2] if c0 == 0 else lq[2:4],
                                ((c0, hw_), (c0 + hw_, cw - hw_))):
                            ENG[q].dma_start(
                                out=out[h, yc * 128 : yc * 128 + 128,
                                        d0 : d0 + dw],
                                in_=ob[:, d0 : d0 + dw])
                        continue
                    dma_splits = ((0, 960), (960, 960)) if fat \
                        else ((0, cw),)
                    for ds, dw in dma_splits:
                        eng = ENG[TAIL_DMA[tail_dma_tick[0]
                                           % len(TAIL_DMA)]]
                        tail_dma_tick[0] += 1
                        eng.dma_start(
                            out=out[h, yc * 128 : yc * 128 + 128,
                                    c0 + ds : c0 + ds + dw],
                            in_=ob[:, c0 + ds : c0 + ds + dw])
                if tail_tile is None:
                    eng = ENG[MID_DMA[mid_dma_tick[0] % len(MID_DMA)]]
                    mid_dma_tick[0] += 1
                    eng.dma_start(
                        out=out[h, yc * 128 : yc * 128 + 128, :], in_=ob)

            # phase 1 per sample in CFG group order (P=Pool, V=DVE
            # subtract); groups are 2 disparities each (d0 = 2*g)
            GRP = [(2 * g, 2) for g in range(12)]
            ORD0 = list(cfg["order0"])
            ORD1 = list(cfg["order1"])
            for i, (g, e) in enumerate(ORD0):
                emit_ph1_group(0, *GRP[g],
                               eng=nc.gpsimd if e == "P" else None,
                               last_g=(i == len(ORD0) - 1))
            g0, e0 = ORD1[0]
            emit_ph1_group(1, *GRP[g0],
                           eng=nc.gpsimd if e0 == "P" else None)
            emit_pred(0)
            tmp0 = emit_ph2_head(0, copy_mid, (mid_tile, mid_tile))
            MR = cfg["mid_rows"]
            mid_done = 0
            for i in range(1, len(ORD1)):
                g, e = ORD1[i]
                emit_ph1_group(1, *GRP[g],
                               eng=nc.gpsimd if e == "P" else None,
                               last_g=(i == len(ORD1) - 1))
                # rows 0..MR-1 of sample 0, spread over the slots
                want = (i * MR) // (len(ORD1) - 1)
                while mid_done < want:
                    emit_ph2_yc(0, tmp0, mid_done, copy_mid)
                    mid_done += 1
            emit_pred(1)
            ph1_stack.close()  # free cost (2) + s/t (2) + out (4) banks
            tail_bufs = {2: 4, 4: 8}[cfg["tail_chunks"]] \
                if not cfg["fat_tail"] else 2
            tail_w = 2048 if cfg["fat_tail"] else \
                {2: 1024, 4: 512}[cfg["tail_chunks"]]
            with tc.tile_pool(name="pstail", bufs=tail_bufs,
                              space="PSUM") as pstail:
                def tail_tile():
                    return pstail.tile([128, tail_w], F32, name="tl",
                                       tag="tl")

                for yc in range(mid_done, 8):
                    emit_ph2_yc(0, tmp0, yc, copy_tail, tail_tile)
                tmp1 = emit_ph2_head(1, copy_tail, (tail_tile, tail_tile),
                                     narrow=(cfg["tail_chunks"] == 4))
                for yc in range(8):
                    emit_ph2_yc(1, tmp1, yc, copy_tail, tail_tile,
                                last_row=(yc >= 8 - cfg["lastrow"]))
    nc.compile()
    return nc


_NC_CACHE = [None]


def kernel(feat_l, feat_r, img_h, img_w):
    feat_l = np.asarray(feat_l, dtype=np.float32)
    feat_r = np.asarray(feat_r, dtype=np.float32)
    assert int(img_h) == HI and int(img_w) == WI
    assert feat_l.shape == (B, C, H0, W0)

    if _NC_CACHE[0] is None:
        _NC_CACHE[0] = _build()
    nc = _NC_CACHE[0]

    consts = _host_consts()
    in_maps = []
    for c in range(NCORES):
        fl = _pack_feat(feat_l[SPC * c : SPC * c + SPC].astype(np.float16))
        fr = _pack_feat_padded(
            feat_r[SPC * c : SPC * c + SPC].astype(np.float16))
        in_maps.append({"lf": fl, "rf": fr, **consts})

    res = run_bass_kernel_spmd(nc, in_maps, core_ids=list(range(NCORES)),
                               trace=_TRACE[0])
    outs = [res.results[i]["out"].astype(np.float32) for i in range(NCORES)]
    full = np.concatenate(outs, axis=0).reshape(B, 1, HI, WI)
    kernel._last_exec_ns = res.exec_time_ns
    return full


# revision 57
# speedup vs baseline: 1.0052x; 1.0052x over previous
"""Stereo cost-volume + softmax disparity regression + bilinear upsample.

Full inputs:  feat_l, feat_r [16, 4, 128, 240] f32, img_h=1024, img_w=1920.
Full output:  [16, 1, 1024, 1920] f32.

Sharding: pure data parallel, 2 samples per core across 8 cores; the two
samples run as a software pipeline (sample 1's cost volume overlaps
sample 0's upsample).

Phase 1 (12 two-disparity groups per sample, double-buffered through two
1-bank PSUM cost tiles so the conveyor never serializes on exp):
  - Subtract |L - R(x-d)| for a whole group in ONE tensor_tensor (custom
    4D access pattern walking the host-pre-padded feat_r window at
    stride 1); groups split Pool/DVE per CFG order strings (Pool's
    serial sub conveyor is the phase-1 pacer).  The u16 bitwise abs
    runs on DVE (4x mode, ~1ns/4el).
  - Channel sum runs "flipped" on the PE: the diff chunk [128, 120] is
    the stationary lhsT and the [128, 32] selector streams, producing
    cost chunks [120(x), 32(y32)].  Layout: cost[x, (sec, xb, yb, y32)],
    xb-major so pred's per-xb M1 weight slices stay contiguous (the BIR
    verifier requires single-free-dim matmul weight APs).
  - ACT exponentiates a whole group tile [120, 512] at once.
  - s/t accumulate in PSUM via TWO fat matmuls per section (s += e,
    t += 8d*e over a whole [120, 256] section; st layout
    [s(xb,yb,y32) | t(xb,yb,y32)]); PSUM lazy-zero semantics allow
    interleaved region accumulation with start exactly once per bank.
Phase 2: pred = t * (1/s) comes out ALREADY x-transposed
  [120(x), (xb, y)], so M1 (x-interp) consumes contiguous [120, 128]
  slices as lhsT -- no PE transposes.  M1 writes one merged
  tmp [128, 1920]; M2 (y-interp) reads arbitrary 512-col slices.
  PSUM->SBUF copies go to ACT and DVE per CFG lane patterns (GPSIMD
  cannot read PSUM; the two copy lanes are the phase-2 pacer).  Mid rows
  (overlapped with s1's phase 1) and tail rows both use 2-bank PSUM
  tiles with two chunk copies per row; output DMAs ride SP/Pool queues,
  with the final rows chunked 4-way to shorten the drain.

Constants are packed into DMA blobs (sel+sid, wyT+wxTa+wxTb; tid on its
own SP slot) to cut descriptor floors.  All engine assignments live in
CFG, tuned by sweeping CoreSim.  Note: nc.vector tensor_tensor divide
and >1-free-dim matmul weight APs pass CoreSim but fail neuronxcc --
keep "divide" off.
"""
import sys

sys.path.insert(0, "/opt/trn_rl_repo")

import numpy as np

import concourse.bass as bass
import concourse.bacc as bacc
import concourse.tile as tile
import concourse.mybir as mybir
from concourse.bass_utils import run_bass_kernel_spmd

# ---------------------------------------------------------------- constants
B, C, H0, W0 = 16, 4, 128, 240
D = 24             # disparities
NCORES = 8
SPC = B // NCORES  # samples per core = 2
HI, WI = 1024, 1920
WP = WI
XB = 120           # x-block width (two blocks per row)
# M2 / output X chunks (PSUM <= 512 cols each)
XCH = [(0, 512), (512, 512), (1024, 512), (1536, 384)]
# M1 X chunks: (start, width, x-halves needed); 956/964 is the exact
# pure-A/pure-B wxT boundary (only an 8-column sliver needs both halves)
XCH_M1 = [(0, 512, (0,)), (512, 444, (0,)), (956, 8, (0, 1)),
          (964, 504, (1,)), (1468, 452, (1,))]
YB = H0 // 32      # 4 y-blocks
G = SPC * YB       # 8 feat groups (sample-major)
FREE = G * W0      # 1920
PAD = 28           # left-pad columns in padded feat_r groups
GW = W0 + 2 * PAD  # padded group width (even)
EXP_BIAS = 8.0

# consts blob1 layout: sel [128,32] | sid [120,120]; tid loads separately
B1_SEL, B1_SID = 0, 32
B1_W = 152
# consts blob2 layout: wyT [128,1024] | wxTa [120,1920] | wxTb [120,1920]
B2_WY, B2_WXA, B2_WXB = 0, HI, HI + WI
B2_W = HI + 2 * WI

FP16 = mybir.dt.float16
F32 = mybir.dt.float32
U16 = mybir.dt.uint16

_TRACE = [False]


# ------------------------------------------------------------- host weights
def _host_consts():
    # selector for the flipped channel sum: sel[ch*32+y32, y'] = (y32 == y')
    sel = np.zeros((128, 32), np.float16)
    for ch in range(C):
        sel[ch * 32 : (ch + 1) * 32, :] = np.eye(32, dtype=np.float16)

    # s identity and per-disparity t identities (8*d scaling)
    sid = np.eye(XB, dtype=np.float16)
    tid = np.zeros((XB, D * XB), np.float16)
    for d in range(D):
        tid[:, d * XB : (d + 1) * XB] = np.eye(XB, dtype=np.float16) * \
            np.float16(8.0 * d)

    # x-interp weights wxT[x, X], f32 linspace to match jnp rounding
    xs = np.linspace(0.0, W0 - 1.0, WI, dtype=np.float32)
    x0 = np.floor(xs).astype(np.int64)
    x1 = np.minimum(x0 + 1, W0 - 1)
    wx = (xs - x0).astype(np.float32)
    wxT_full = np.zeros((W0, WI), np.float32)
    wxT_full[x0, np.arange(WI)] += 1.0 - wx
    wxT_full[x1, np.arange(WI)] += wx
    # chunk validity: columns left of 956 only use x<120; right of 964 only
    # x>=120; the 8-col sliver uses both
    assert x1[:956].max() <= XB - 1
    assert x0[964:].min() >= XB
    wxTa = wxT_full[0:XB]
    wxTb = wxT_full[XB : 2 * XB]

    # y-interp weights wyT[y, Y]
    ys = np.linspace(0.0, H0 - 1.0, HI, dtype=np.float32)
    y0 = np.floor(ys).astype(np.int64)
    y1 = np.minimum(y0 + 1, H0 - 1)
    wy = (ys - y0).astype(np.float32)
    wyT = np.zeros((H0, HI), np.float32)
    wyT[y0, np.arange(HI)] += 1.0 - wy
    wyT[y1, np.arange(HI)] += wy

    cst1 = np.zeros((128, B1_W), np.float16)
    cst1[:, B1_SEL:B1_SID] = sel
    cst1[0:XB, B1_SID:B1_W] = sid
    cst2 = np.zeros((128, B2_W), np.float16)
    cst2[:, B2_WY:B2_WXA] = wyT.astype(np.float16)
    cst2[0:XB, B2_WXA:B2_WXB] = wxTa.astype(np.float16)
    cst2[0:XB, B2_WXB:B2_W] = wxTb.astype(np.float16)
    return {"cst1": cst1, "cst2": cst2, "ctid": tid}


def _pack_feat(f):
    """[SPC, C, H0, W0] -> [128, FREE] with p=(ch,y32), free=(s,yb,x)."""
    a = f.reshape(SPC, C, YB, 32, W0)
    a = np.ascontiguousarray(a.transpose(1, 3, 0, 2, 4))  # ch,y32,s,yb,x
    return a.reshape(128, FREE)


def _pack_feat_padded(f):
    """[SPC, C, H0, W0] -> [128, SPC*YB*GW], PAD zero cols around each row."""
    a = f.reshape(SPC, C, YB, 32, W0).transpose(1, 3, 0, 2, 4)
    p = np.zeros((C, 32, SPC, YB, GW), f.dtype)
    p[:, :, :, :, PAD : PAD + W0] = a
    return p.reshape(128, SPC * YB * GW)


# scheduling configuration (engine assignment knobs, tuned via sweep).
# orderN: per-sample phase-1 emission order of the 12 two-disparity
# groups, each tagged with its subtract engine (P=Pool, V=DVE).
_O = "VPVPPVPPVPPP"
CFG = {
    "order0": tuple((i, _O[i]) for i in range(12)),
    "order1": tuple((i, _O[i]) for i in range(12)),
    "mid_pat": "AV",           # PSUM->SBUF copy lanes, mid rows
    "tail_pat": "AV",          # tail rows
    "mid_dma": "S",            # output DMA queues, mid rows (full row)
    "tail_dma": "PS",          # tail half-row DMA queue pattern
    "mid_rows": 6,             # s0 rows emitted during s1 phase 1
    "lf0_pool": False,         # load lf sample-0 half on Pool's queue
    "lastrow": 2,              # trailing rows with 4-way chunked DMA
    "divide": False,            # pred via DVE divide (vs recip+mult)
    "fat_mid": False,          # mid rows: one 4-bank tile + one fat copy
    "fat_tail": False,         # tail rows: 4-bank tiles + one fat copy
    "tail_chunks": 2,          # tail row chunking: 2x1024 or 4x512
    "lr_q": "SPPS",            # queues for the 4 chunked last-row DMAs
    "ob_bufs": 6,              # output staging tiles in SBUF
}


CFG_OB = [6]


# ------------------------------------------------------------- build kernel
def _build(cfg=None):
    cfg = {**CFG, **(cfg or {})}
    CFG_OB[0] = cfg.get("ob_bufs", 6)
    nc = bacc.Bacc("TRN2", target_bir_lowering=False, debug=False,
                   num_devices=NCORES)
    lf = nc.dram_tensor("lf", [128, FREE], FP16, kind="ExternalInput").ap()
    rf = nc.dram_tensor("rf", [128, SPC * YB * GW], FP16,
                        kind="ExternalInput").ap()
    cst1_d = nc.dram_tensor("cst1", [128, B1_W], FP16,
                            kind="ExternalInput").ap()
    ctid_d = nc.dram_tensor("ctid", [XB, D * XB], FP16,
                            kind="ExternalInput").ap()
    cst2_d = nc.dram_tensor("cst2", [128, B2_W], FP16,
                            kind="ExternalInput").ap()
    out = nc.dram_tensor("out", [SPC, HI, WI], FP16,
                         kind="ExternalOutput").ap()

    AF = mybir.ActivationFunctionType
    OP = mybir.AluOpType

    with tile.TileContext(nc) as tc:
        with (
            tc.tile_pool(name="consts", bufs=1) as consts,
            tc.tile_pool(name="feat", bufs=1) as feat,
            tc.tile_pool(name="diff", bufs=8) as diffp,
            tc.tile_pool(name="ep", bufs=6) as ep,
            tc.tile_pool(name="predp", bufs=1) as predp,
            tc.tile_pool(name="upsb", bufs=1) as upsb,
            tc.tile_pool(name="outsb", bufs=CFG_OB[0]) as outsb,
        ):
            from contextlib import ExitStack
            ph1_stack = ExitStack()
            # PSUM budget during phase 1 + mid: cost 2x1 bank (double-
            # buffered 2-disp groups) + st0/st1 (2) + two 2-bank output
            # tiles (4) = 8.
            costp = ph1_stack.enter_context(
                tc.tile_pool(name="costp", bufs=2, space="PSUM"))
            stps = ph1_stack.enter_context(
                tc.tile_pool(name="stps", bufs=1, space="PSUM"))
            outps = ph1_stack.enter_context(
                tc.tile_pool(name="outps",
                             bufs=1 if cfg["fat_mid"] else 2,
                             space="PSUM"))

            bias8 = consts.tile([XB, 1], F32)
            nc.vector.memset(bias8, EXP_BIAS)

            # ---- input DMAs spread across the four queues so everything
            # lands early: lf halves + blob2 on SP, rf h0 on Pool (feeds
            # Pool's first subtract), rf h1 on DVE (idle at start), the
            # phase-1 consts blob on ACT.
            # SP queue order: lf-s0, cb1 (needed ~5us), rf-s1, lf-s1, cb2
            # (needed at M1 ~15us).  rf-s0 rides Pool's own queue so its
            # first subtract follows immediately.
            Lh, R = [], [None, None]
            Rt0 = feat.tile([128, YB * GW], FP16, tag="rpad0", name="rpad0")
            nc.gpsimd.dma_start(out=Rt0, in_=rf[:, 0 : YB * GW])
            Lt0 = feat.tile([128, FREE // 2], FP16, tag="L0", name="L0")
            lf0_eng = nc.gpsimd if cfg["lf0_pool"] else nc.sync
            lf0_eng.dma_start(out=Lt0, in_=lf[:, 0 : FREE // 2])
            cb1 = consts.tile([128, B1_W], FP16, name="cb1", tag="cb1")
            nc.scalar.dma_start(out=cb1, in_=cst1_d)
            tid = consts.tile([XB, D * XB], FP16, name="tid", tag="tid")
            nc.sync.dma_start(out=tid, in_=ctid_d)
            Rt1 = feat.tile([128, YB * GW], FP16, tag="rpad1", name="rpad1")
            nc.sync.dma_start(out=Rt1, in_=rf[:, YB * GW : 2 * YB * GW])
            Lt1 = feat.tile([128, FREE // 2], FP16, tag="L1", name="L1")
            nc.sync.dma_start(out=Lt1, in_=lf[:, FREE // 2 : FREE])
            cb2 = consts.tile([128, B2_W], FP16, name="cb2", tag="cb2")
            nc.sync.dma_start(out=cb2, in_=cst2_d)
            for Lt in (Lt0, Lt1):
                Lh.append(Lt.rearrange("p (g w) -> p g w", w=W0))
            for h2, Rt in enumerate((Rt0, Rt1)):
                R[h2] = Rt.rearrange("p (g w) -> p g w", w=GW)

            sel = cb1[:, B1_SEL:B1_SID]
            sid = cb1[0:XB, B1_SID:B1_W]
            wyT = cb2[:, B2_WY:B2_WXA]
            wxT = [cb2[0:XB, B2_WXA:B2_WXB], cb2[0:XB, B2_WXB:B2_W]]

            st = [stps.tile([XB, 512], F32, name=f"st{h}", tag=f"st{h}")
                  for h in range(SPC)]

            # ---------- copy lanes
            mid_tick = [0]
            tail_tick = [0]

            def _copy_on(eng, dst, src):
                # PSUM->SBUF moves: ACT or DVE only (GPSIMD cannot touch
                # PSUM -- the BIR verifier rejects it)
                if eng == "A":
                    nc.scalar.copy(out=dst, in_=src)
                else:
                    nc.vector.tensor_copy(out=dst, in_=src)

            MID_PAT = list(cfg["mid_pat"])
            TAIL_PAT = list(cfg["tail_pat"])
            MID_DMA = list(cfg["mid_dma"])
            TAIL_DMA = list(cfg["tail_dma"])
            ENG = {"S": nc.sync, "A": nc.scalar, "P": nc.gpsimd,
                   "V": nc.vector}

            def copy_mid(dst, src):
                _copy_on(MID_PAT[mid_tick[0] % len(MID_PAT)], dst, src)
                mid_tick[0] += 1

            def copy_tail(dst, src):
                _copy_on(TAIL_PAT[tail_tick[0] % len(TAIL_PAT)], dst, src)
                tail_tick[0] += 1

            # ============ software pipeline over the two samples =========
            pred = [None, None]
            # diff tile sections hold disparities hi-first: [d+3,d+2,d+1,d]
            st_open = [False, False]

            def emit_ph1_group(h, d0, nsec, eng=None, last_g=False):
                eng = eng or nc.vector
                Dt = diffp.tile([128, 2 * YB * W0], FP16, name="diff",
                                tag="diff")
                D4 = Dt.rearrange("p (s g w) -> p s g w", s=2, w=W0)[:, 0:nsec]
                Lk = Lh[h].unsqueeze(1).broadcast_to([128, nsec, YB, W0])
                # one subtract covers disparities d0+nsec-1..d0 via a k-dim
                # stepping the feat_r window right by 1
                off_hi = PAD - (d0 + nsec - 1)
                Rbase = R[h][:, :, off_hi : off_hi + W0]
                Rk = bass.AP(
                    Rbase.tensor, Rbase.offset,
                    [list(Rbase.ap[0]), [1, nsec],
                     list(Rbase.ap[1]), list(Rbase.ap[2])])
                eng.tensor_tensor(out=D4, in0=Lk, in1=Rk, op=OP.subtract)
                Du = Dt.bitcast(U16)[:, 0 : nsec * YB * W0]
                nc.vector.tensor_scalar(
                    out=Du, in0=Du, scalar1=0x7FFF, scalar2=None,
                    op0=OP.bitwise_and,
                )
                D3 = Dt.rearrange("p (s f) -> p s f", s=2)
                # flipped channel sum: cost[x, (sec, xb, yb, y32)] --
                # xb-major so pred's per-xb M1 weight slices are contiguous
                cost = costp.tile([XB, 512], F32, name="cost", tag="cost")
                e = ep.tile([XB, 512], FP16, name="e", tag="e")
                for sec in range(nsec):
                    for yb in range(YB):
                        for xb in range(2):
                            nc.tensor.matmul(
                                cost[0:XB,
                                     sec * 256 + xb * 128 + yb * 32 :
                                     sec * 256 + xb * 128 + yb * 32 + 32],
                                lhsT=D3[:, sec,
                                        yb * W0 + xb * XB :
                                        yb * W0 + xb * XB + XB],
                                rhs=sel,
                                start=(sec == 0 and yb == 0 and xb == 0),
                                stop=(sec == nsec - 1
                                      and yb == YB - 1 and xb == 1),
                                skip_group_check=True,
                            )
                nc.scalar.activation(out=e[:, 0 : nsec * 256],
                                     in_=cost[:, 0 : nsec * 256],
                                     func=AF.Exp, bias=bias8, scale=-1.0)
                for sec in range(nsec):
                    _emit_st_sec(h, d0 + (nsec - 1 - sec), sec, e, last_g
                                 and sec == nsec - 1)

            def _emit_st_sec(h, d, sec, e, last_sec):
                # fat accumulate over a whole [120, 256] section:
                # st layout [s(yb,xb,y32) | t(yb,xb,y32)] matches e's order
                first = not st_open[h]
                st_open[h] = True
                rhs = e[:, sec * 256 : sec * 256 + 256]
                nc.tensor.matmul(
                    st[h][0:XB, 0:256], lhsT=sid, rhs=rhs,
                    start=first, stop=False, skip_group_check=True,
                )
                nc.tensor.matmul(
                    st[h][0:XB, 256:512],
                    lhsT=tid[:, d * XB : d * XB + XB], rhs=rhs,
                    start=False, stop=last_sec, skip_group_check=True,
                )

            def emit_pred(h):
                pr = predp.tile([XB, 256], FP16, name=f"pred{h}",
                                tag=f"pred{h}")
                if cfg["divide"]:
                    nc.vector.tensor_tensor(out=pr, in0=st[h][:, 256:512],
                                            in1=st[h][:, 0:256],
                                            op=OP.divide)
                else:
                    rs = predp.tile([XB, 256], F32, name=f"rs{h}",
                                    tag=f"rs{h}")
                    nc.vector.reciprocal(out=rs, in_=st[h][:, 0:256])
                    nc.vector.tensor_tensor(out=pr, in0=st[h][:, 256:512],
                                            in1=rs, op=OP.mult)
                # pred layout [x, (xb, yb, y32)] = [x, (xb, y128)]
                pred[h] = pr

            def mid_tile():
                if cfg["fat_mid"]:
                    return outps.tile([128, 2048], F32, name="o_ps",
                                      tag="o_ps")
                return outps.tile([128, 1024], F32, name="o_ps",
                                  tag="o_ps")

            def emit_ph2_head(h, copy_fn, pool_fns, narrow=False):
                """M1 -> merged tmp [128, WP].  narrow: per-chunk tiles
                (<=512 wide); else two fat half copies from >=2-bank
                tiles."""
                lhs = [pred[h][:, xb * 128 : xb * 128 + 128]
                       for xb in range(2)]
                tmp = upsb.tile([128, WP], FP16, tag=f"tmp{h}",
                                name=f"tmp{h}")
                if narrow:
                    for c0, nw, halves in XCH_M1:
                        t_ps = pool_fns[0]()
                        for i, xb in enumerate(halves):
                            nc.tensor.matmul(
                                t_ps[:, 0:nw], lhsT=lhs[xb],
                                rhs=wxT[xb][:, c0 : c0 + nw],
                                start=(i == 0), stop=(i == len(halves) - 1),
                            )
                        copy_fn(tmp[:, c0 : c0 + nw], t_ps[:, 0:nw])
                    return tmp
                for half, (lo, hi, base, wtot) in enumerate(
                        ((0, 2, 0, 956), (2, 5, 956, WP - 956))):
                    t_ps = pool_fns[half]()
                    for c0, nw, halves in XCH_M1[lo:hi]:
                        for i, xb in enumerate(halves):
                            nc.tensor.matmul(
                                t_ps[:, c0 - base : c0 - base + nw],
                                lhsT=lhs[xb],
                                rhs=wxT[xb][:, c0 : c0 + nw],
                                start=(i == 0),
                                stop=(i == len(halves) - 1),
                            )
                    copy_fn(tmp[:, base : base + wtot], t_ps[:, 0:wtot])
                return tmp

            mid_dma_tick = [0]
            tail_dma_tick = [0]

            def emit_ph2_yc(h, tmp, yc, copy_fn, tail_tile=None,
                            last_row=False):
                """tail_tile=None: two 2-bank chunks + one full-row DMA
                (mid rows).  tail_tile: 2-bank tiles, one copy + one DMA
                per half-row (tail).  last_row: 4-way chunked DMA across
                queues to shorten the final drain."""
                ob = outsb.tile([128, WP], FP16, name="ob", tag="ob")
                wy_col = wyT[:, yc * 128 : yc * 128 + 128]
                tile_fn = tail_tile or mid_tile
                fat = cfg["fat_tail"] if tail_tile else cfg["fat_mid"]
                if tail_tile is not None and cfg["tail_chunks"] == 4:
                    chunks = tuple(XCH)
                elif fat:
                    chunks = ((0, WP),)
                else:
                    chunks = ((0, 1024), (1024, WP - 1024))
                for c0, cw in chunks:
                    o_ps = tile_fn()
                    for xc0, xnw in XCH:
                        if xc0 < c0 or xc0 >= c0 + cw:
                            continue
                        nc.tensor.matmul(
                            o_ps[:, xc0 - c0 : xc0 - c0 + xnw],
                            lhsT=wy_col, rhs=tmp[:, xc0 : xc0 + xnw],
                            start=True, stop=True,
                        )
                    copy_fn(ob[:, c0 : c0 + cw], o_ps[:, 0:cw])
                    if tail_tile is None:
                        continue
                    if last_row:
                        hw_ = cw // 2
                        lq = cfg["lr_q"]
                        for q, (d0, dw) in zip(
                                lq[0:2] if c0 == 0 else lq[2:4],
                                ((c0, hw_), (c0 + hw_, cw - hw_))):
                            ENG[q].dma_start(
                                out=out[h, yc * 128 : yc * 128 + 128,
                                        d0 : d0 + dw],
                                in_=ob[:, d0 : d0 + dw])
                        continue
                    dma_splits = ((0, 960), (960, 960)) if fat \
                        else ((0, cw),)
                    for ds, dw in dma_splits:
                        eng = ENG[TAIL_DMA[tail_dma_tick[0]
                                           % len(TAIL_DMA)]]
                        tail_dma_tick[0] += 1
                        eng.dma_start(
                            out=out[h, yc * 128 : yc * 128 + 128,
                                    c0 + ds : c0 + ds + dw],
                            in_=ob[:, c0 + ds : c0 + ds + dw])
                if tail_tile is None:
                    eng = ENG[MID_DMA[mid_dma_tick[0] % len(MID_DMA)]]
                    mid_dma_tick[0] += 1
                    eng.dma_start(
                        out=out[h, yc * 128 : yc * 128 + 128, :], in_=ob)

            # phase 1 per sample in CFG group order (P=Pool, V=DVE
            # subtract); groups are 2 disparities each (d0 = 2*g)
            GRP = [(2 * g, 2) for g in range(12)]
            ORD0 = list(cfg["order0"])
            ORD1 = list(cfg["order1"])
            for i, (g, e) in enumerate(ORD0):
                emit_ph1_group(0, *GRP[g],
                               eng=nc.gpsimd if e == "P" else None,
                               last_g=(i == len(ORD0) - 1))
            g0, e0 = ORD1[0]
            emit_ph1_group(1, *GRP[g0],
                           eng=nc.gpsimd if e0 == "P" else None)
            emit_pred(0)
            tmp0 = emit_ph2_head(0, copy_mid, (mid_tile, mid_tile))
            MR = cfg["mid_rows"]
            mid_done = 0
            for i in range(1, len(ORD1)):
                g, e = ORD1[i]
                emit_ph1_group(1, *GRP[g],
                               eng=nc.gpsimd if e == "P" else None,
                               last_g=(i == len(ORD1) - 1))
                # rows 0..MR-1 of sample 0, spread over the slots
                want = (i * MR) // (len(ORD1) - 1)
                while mid_done < want:
                    emit_ph2_yc(0, tmp0, mid_done, copy_mid)
                    mid_done += 1
            emit_pred(1)
            ph1_stack.close()  # free cost (2) + s/t (2) + out (4) banks
            tail_bufs = {2: 4, 4: 8}[cfg["tail_chunks"]] \
                if not cfg["fat_tail"] else 2
            tail_w = 2048 if cfg["fat_tail"] else \
                {2: 1024, 4: 512}[cfg["tail_chunks"]]
            with tc.tile_pool(name="pstail", bufs=tail_bufs,
                              space="PSUM") as pstail:
                def tail_tile():
                    return pstail.tile([128, tail_w], F32, name="tl",
                                       tag="tl")

                for yc in range(mid_done, 8):
                    emit_ph2_yc(0, tmp0, yc, copy_tail, tail_tile)
                tmp1 = emit_ph2_head(1, copy_tail, (tail_tile, tail_tile),
                                     narrow=(cfg["tail_chunks"] == 4))
                for yc in range(8):
                    emit_ph2_yc(1, tmp1, yc, copy_tail, tail_tile,
                                last_row=(yc >= 8 - cfg["lastrow"]))
    nc.compile()
    return nc


_NC_CACHE = [None]


def kernel(feat_l, feat_r, img_h, img_w):
    feat_l = np.asarray(feat_l, dtype=np.float32)
    feat_r = np.asarray(feat_r, dtype=np.float32)
    assert int(img_h) == HI and int(img_w) == WI
    assert feat_l.shape == (B, C, H0, W0)

    if _NC_CACHE[0] is None:
        _NC_CACHE[0] = _build()
    nc = _NC_CACHE[0]

    consts = _host_consts()
    in_maps = []
    for c in range(NCORES):
        fl = _pack_feat(feat_l[SPC * c : SPC * c + SPC].astype(np.float16))
        fr = _pack_feat_padded(
            feat_r[SPC * c : SPC * c + SPC].astype(np.float16))
        in_maps.append({"lf": fl, "rf": fr, **consts})

    res = run_bass_kernel_spmd(nc, in_maps, core_ids=list(range(NCORES)),
                               trace=_TRACE[0])
    outs = [res.results[i]["out"].astype(np.float32) for i in range(NCORES)]
    full = np.concatenate(outs, axis=0).reshape(B, 1, HI, WI)
    kernel._last_exec_ns = res.exec_time_ns
    return full


# revision 58
# speedup vs baseline: 1.0053x; 1.0002x over previous
"""Stereo cost-volume + softmax disparity regression + bilinear upsample.

Full inputs:  feat_l, feat_r [16, 4, 128, 240] f32, img_h=1024, img_w=1920.
Full output:  [16, 1, 1024, 1920] f32.

Sharding: pure data parallel, 2 samples per core across 8 cores; the two
samples run as a software pipeline (sample 1's cost volume overlaps
sample 0's upsample).

Phase 1 (12 two-disparity groups per sample, double-buffered through two
1-bank PSUM cost tiles so the conveyor never serializes on exp):
  - Subtract |L - R(x-d)| for a whole group in ONE tensor_tensor (custom
    4D access pattern walking the host-pre-padded feat_r window at
    stride 1); groups split Pool/DVE per CFG order strings (Pool's
    serial sub conveyor is the phase-1 pacer).  The u16 bitwise abs
    runs on DVE (4x mode, ~1ns/4el).
  - Channel sum runs "flipped" on the PE: the diff chunk [128, 120] is
    the stationary lhsT and the [128, 32] selector streams, producing
    cost chunks [120(x), 32(y32)].  Layout: cost[x, (sec, xb, yb, y32)],
    xb-major so pred's per-xb M1 weight slices stay contiguous (the BIR
    verifier requires single-free-dim matmul weight APs).
  - ACT exponentiates a whole group tile [120, 512] at once.
  - s/t accumulate in PSUM via TWO fat matmuls per section (s += e,
    t += 8d*e over a whole [120, 256] section; st layout
    [s(xb,yb,y32) | t(xb,yb,y32)]); PSUM lazy-zero semantics allow
    interleaved region accumulation with start exactly once per bank.
Phase 2: pred = t * (1/s) comes out ALREADY x-transposed
  [120(x), (xb, y)], so M1 (x-interp) consumes contiguous [120, 128]
  slices as lhsT -- no PE transposes.  M1 writes one merged
  tmp [128, 1920]; M2 (y-interp) reads arbitrary 512-col slices.
  PSUM->SBUF copies go to ACT and DVE per CFG lane patterns (GPSIMD
  cannot read PSUM; the two copy lanes are the phase-2 pacer).  Mid rows
  (overlapped with s1's phase 1) and tail rows both use 2-bank PSUM
  tiles with two chunk copies per row; output DMAs ride SP/Pool queues,
  with the final rows chunked 4-way to shorten the drain.

Constants are packed into DMA blobs (sel+sid, wyT+wxTa+wxTb; tid on its
own SP slot) to cut descriptor floors.  All engine assignments live in
CFG, tuned by sweeping CoreSim.  Note: nc.vector tensor_tensor divide
and >1-free-dim matmul weight APs pass CoreSim but fail neuronxcc --
keep "divide" off.
"""
import sys

sys.path.insert(0, "/opt/trn_rl_repo")

import numpy as np

import concourse.bass as bass
import concourse.bacc as bacc
import concourse.tile as tile
import concourse.mybir as mybir
from concourse.bass_utils import run_bass_kernel_spmd

# ---------------------------------------------------------------- constants
B, C, H0, W0 = 16, 4, 128, 240
D = 24             # disparities
NCORES = 8
SPC = B // NCORES  # samples per core = 2
HI, WI = 1024, 1920
WP = WI
XB = 120           # x-block width (two blocks per row)
# M2 / output X chunks (PSUM <= 512 cols each)
XCH = [(0, 512), (512, 512), (1024, 512), (1536, 384)]
# M1 X chunks: (start, width, x-halves needed); 956/964 is the exact
# pure-A/pure-B wxT boundary (only an 8-column sliver needs both halves)
XCH_M1 = [(0, 512, (0,)), (512, 444, (0,)), (956, 8, (0, 1)),
          (964, 504, (1,)), (1468, 452, (1,))]
YB = H0 // 32      # 4 y-blocks
G = SPC * YB       # 8 feat groups (sample-major)
FREE = G * W0      # 1920
PAD = 28           # left-pad columns in padded feat_r groups
GW = W0 + 2 * PAD  # padded group width (even)
EXP_BIAS = 8.0

# consts blob1 layout: sel [128,32] | sid [120,120]; tid loads separately
B1_SEL, B1_SID = 0, 32
B1_W = 152
# consts blob2 layout: wyT [128,1024] | wxTa [120,1920] | wxTb [120,1920]
B2_WY, B2_WXA, B2_WXB = 0, HI, HI + WI
B2_W = HI + 2 * WI

FP16 = mybir.dt.float16
F32 = mybir.dt.float32
U16 = mybir.dt.uint16

_TRACE = [False]


# ------------------------------------------------------------- host weights
def _host_consts():
    # selector for the flipped channel sum: sel[ch*32+y32, y'] = (y32 == y')
    sel = np.zeros((128, 32), np.float16)
    for ch in range(C):
        sel[ch * 32 : (ch + 1) * 32, :] = np.eye(32, dtype=np.float16)

    # s identity and per-disparity t identities (8*d scaling)
    sid = np.eye(XB, dtype=np.float16)
    tid = np.zeros((XB, D * XB), np.float16)
    for d in range(D):
        tid[:, d * XB : (d + 1) * XB] = np.eye(XB, dtype=np.float16) * \
            np.float16(8.0 * d)

    # x-interp weights wxT[x, X], f32 linspace to match jnp rounding
    xs = np.linspace(0.0, W0 - 1.0, WI, dtype=np.float32)
    x0 = np.floor(xs).astype(np.int64)
    x1 = np.minimum(x0 + 1, W0 - 1)
    wx = (xs - x0).astype(np.float32)
    wxT_full = np.zeros((W0, WI), np.float32)
    wxT_full[x0, np.arange(WI)] += 1.0 - wx
    wxT_full[x1, np.arange(WI)] += wx
    # chunk validity: columns left of 956 only use x<120; right of 964 only
    # x>=120; the 8-col sliver uses both
    assert x1[:956].max() <= XB - 1
    assert x0[964:].min() >= XB
    wxTa = wxT_full[0:XB]
    wxTb = wxT_full[XB : 2 * XB]

    # y-interp weights wyT[y, Y]
    ys = np.linspace(0.0, H0 - 1.0, HI, dtype=np.float32)
    y0 = np.floor(ys).astype(np.int64)
    y1 = np.minimum(y0 + 1, H0 - 1)
    wy = (ys - y0).astype(np.float32)
    wyT = np.zeros((H0, HI), np.float32)
    wyT[y0, np.arange(HI)] += 1.0 - wy
    wyT[y1, np.arange(HI)] += wy

    cst1 = np.zeros((128, B1_W), np.float16)
    cst1[:, B1_SEL:B1_SID] = sel
    cst1[0:XB, B1_SID:B1_W] = sid
    cst2 = np.zeros((128, B2_W), np.float16)
    cst2[:, B2_WY:B2_WXA] = wyT.astype(np.float16)
    cst2[0:XB, B2_WXA:B2_WXB] = wxTa.astype(np.float16)
    cst2[0:XB, B2_WXB:B2_W] = wxTb.astype(np.float16)
    return {"cst1": cst1, "cst2": cst2, "ctid": tid}


def _pack_feat(f):
    """[SPC, C, H0, W0] -> [128, FREE] with p=(ch,y32), free=(s,yb,x)."""
    a = f.reshape(SPC, C, YB, 32, W0)
    a = np.ascontiguousarray(a.transpose(1, 3, 0, 2, 4))  # ch,y32,s,yb,x
    return a.reshape(128, FREE)


def _pack_feat_padded(f):
    """[SPC, C, H0, W0] -> [128, SPC*YB*GW], PAD zero cols around each row."""
    a = f.reshape(SPC, C, YB, 32, W0).transpose(1, 3, 0, 2, 4)
    p = np.zeros((C, 32, SPC, YB, GW), f.dtype)
    p[:, :, :, :, PAD : PAD + W0] = a
    return p.reshape(128, SPC * YB * GW)


# scheduling configuration (engine assignment knobs, tuned via sweep).
# orderN: per-sample phase-1 emission order of the 12 two-disparity
# groups, each tagged with its subtract engine (P=Pool, V=DVE).
_O = "VPVPPVPPVPPP"
CFG = {
    "order0": tuple((i, _O[i]) for i in range(12)),
    "order1": tuple((i, _O[i]) for i in range(12)),
    "mid_pat": "AV",           # PSUM->SBUF copy lanes, mid rows
    "tail_pat": "AV",          # tail rows
    "mid_dma": "S",            # output DMA queues, mid rows (full row)
    "tail_dma": "PS",          # tail half-row DMA queue pattern
    "mid_rows": 6,             # s0 rows emitted during s1 phase 1
    "lf0_pool": False,         # load lf sample-0 half on Pool's queue
    "lastrow": 3,              # trailing rows with 4-way chunked DMA
    "divide": False,            # pred via DVE divide (vs recip+mult)
    "fat_mid": False,          # mid rows: one 4-bank tile + one fat copy
    "fat_tail": False,         # tail rows: 4-bank tiles + one fat copy
    "tail_chunks": 2,          # tail row chunking: 2x1024 or 4x512
    "lr_q": "SPPS",            # queues for the 4 chunked last-row DMAs
    "ob_bufs": 6,              # output staging tiles in SBUF
}


CFG_OB = [6]


# ------------------------------------------------------------- build kernel
def _build(cfg=None):
    cfg = {**CFG, **(cfg or {})}
    CFG_OB[0] = cfg.get("ob_bufs", 6)
    nc = bacc.Bacc("TRN2", target_bir_lowering=False, debug=False,
                   num_devices=NCORES)
    lf = nc.dram_tensor("lf", [128, FREE], FP16, kind="ExternalInput").ap()
    rf = nc.dram_tensor("rf", [128, SPC * YB * GW], FP16,
                        kind="ExternalInput").ap()
    cst1_d = nc.dram_tensor("cst1", [128, B1_W], FP16,
                            kind="ExternalInput").ap()
    ctid_d = nc.dram_tensor("ctid", [XB, D * XB], FP16,
                            kind="ExternalInput").ap()
    cst2_d = nc.dram_tensor("cst2", [128, B2_W], FP16,
                            kind="ExternalInput").ap()
    out = nc.dram_tensor("out", [SPC, HI, WI], FP16,
                         kind="ExternalOutput").ap()

    AF = mybir.ActivationFunctionType
    OP = mybir.AluOpType

    with tile.TileContext(nc) as tc:
        with (
            tc.tile_pool(name="consts", bufs=1) as consts,
            tc.tile_pool(name="feat", bufs=1) as feat,
            tc.tile_pool(name="diff", bufs=8) as diffp,
            tc.tile_pool(name="ep", bufs=6) as ep,
            tc.tile_pool(name="predp", bufs=1) as predp,
            tc.tile_pool(name="upsb", bufs=1) as upsb,
            tc.tile_pool(name="outsb", bufs=CFG_OB[0]) as outsb,
        ):
            from contextlib import ExitStack
            ph1_stack = ExitStack()
            # PSUM budget during phase 1 + mid: cost 2x1 bank (double-
            # buffered 2-disp groups) + st0/st1 (2) + two 2-bank output
            # tiles (4) = 8.
            costp = ph1_stack.enter_context(
                tc.tile_pool(name="costp", bufs=2, space="PSUM"))
            stps = ph1_stack.enter_context(
                tc.tile_pool(name="stps", bufs=1, space="PSUM"))
            outps = ph1_stack.enter_context(
                tc.tile_pool(name="outps",
                             bufs=1 if cfg["fat_mid"] else 2,
                             space="PSUM"))

            bias8 = consts.tile([XB, 1], F32)
            nc.vector.memset(bias8, EXP_BIAS)

            # ---- input DMAs spread across the four queues so everything
            # lands early: lf halves + blob2 on SP, rf h0 on Pool (feeds
            # Pool's first subtract), rf h1 on DVE (idle at start), the
            # phase-1 consts blob on ACT.
            # SP queue order: lf-s0, cb1 (needed ~5us), rf-s1, lf-s1, cb2
            # (needed at M1 ~15us).  rf-s0 rides Pool's own queue so its
            # first subtract follows immediately.
            Lh, R = [], [None, None]
            Rt0 = feat.tile([128, YB * GW], FP16, tag="rpad0", name="rpad0")
            nc.gpsimd.dma_start(out=Rt0, in_=rf[:, 0 : YB * GW])
            Lt0 = feat.tile([128, FREE // 2], FP16, tag="L0", name="L0")
            lf0_eng = nc.gpsimd if cfg["lf0_pool"] else nc.sync
            lf0_eng.dma_start(out=Lt0, in_=lf[:, 0 : FREE // 2])
            cb1 = consts.tile([128, B1_W], FP16, name="cb1", tag="cb1")
            nc.scalar.dma_start(out=cb1, in_=cst1_d)
            tid = consts.tile([XB, D * XB], FP16, name="tid", tag="tid")
            nc.sync.dma_start(out=tid, in_=ctid_d)
            Rt1 = feat.tile([128, YB * GW], FP16, tag="rpad1", name="rpad1")
            nc.sync.dma_start(out=Rt1, in_=rf[:, YB * GW : 2 * YB * GW])
            Lt1 = feat.tile([128, FREE // 2], FP16, tag="L1", name="L1")
            nc.sync.dma_start(out=Lt1, in_=lf[:, FREE // 2 : FREE])
            cb2 = consts.tile([128, B2_W], FP16, name="cb2", tag="cb2")
            nc.sync.dma_start(out=cb2, in_=cst2_d)
            for Lt in (Lt0, Lt1):
                Lh.append(Lt.rearrange("p (g w) -> p g w", w=W0))
            for h2, Rt in enumerate((Rt0, Rt1)):
                R[h2] = Rt.rearrange("p (g w) -> p g w", w=GW)

            sel = cb1[:, B1_SEL:B1_SID]
            sid = cb1[0:XB, B1_SID:B1_W]
            wyT = cb2[:, B2_WY:B2_WXA]
            wxT = [cb2[0:XB, B2_WXA:B2_WXB], cb2[0:XB, B2_WXB:B2_W]]

            st = [stps.tile([XB, 512], F32, name=f"st{h}", tag=f"st{h}")
                  for h in range(SPC)]

            # ---------- copy lanes
            mid_tick = [0]
            tail_tick = [0]

            def _copy_on(eng, dst, src):
                # PSUM->SBUF moves: ACT or DVE only (GPSIMD cannot touch
                # PSUM -- the BIR verifier rejects it)
                if eng == "A":
                    nc.scalar.copy(out=dst, in_=src)
                else:
                    nc.vector.tensor_copy(out=dst, in_=src)

            MID_PAT = list(cfg["mid_pat"])
            TAIL_PAT = list(cfg["tail_pat"])
            MID_DMA = list(cfg["mid_dma"])
            TAIL_DMA = list(cfg["tail_dma"])
            ENG = {"S": nc.sync, "A": nc.scalar, "P": nc.gpsimd,
                   "V": nc.vector}

            def copy_mid(dst, src):
                _copy_on(MID_PAT[mid_tick[0] % len(MID_PAT)], dst, src)
                mid_tick[0] += 1

            def copy_tail(dst, src):
                _copy_on(TAIL_PAT[tail_tick[0] % len(TAIL_PAT)], dst, src)
                tail_tick[0] += 1

            # ============ software pipeline over the two samples =========
            pred = [None, None]
            # diff tile sections hold disparities hi-first: [d+3,d+2,d+1,d]
            st_open = [False, False]

            def emit_ph1_group(h, d0, nsec, eng=None, last_g=False):
                eng = eng or nc.vector
                Dt = diffp.tile([128, 2 * YB * W0], FP16, name="diff",
                                tag="diff")
                D4 = Dt.rearrange("p (s g w) -> p s g w", s=2, w=W0)[:, 0:nsec]
                Lk = Lh[h].unsqueeze(1).broadcast_to([128, nsec, YB, W0])
                # one subtract covers disparities d0+nsec-1..d0 via a k-dim
                # stepping the feat_r window right by 1
                off_hi = PAD - (d0 + nsec - 1)
                Rbase = R[h][:, :, off_hi : off_hi + W0]
                Rk = bass.AP(
                    Rbase.tensor, Rbase.offset,
                    [list(Rbase.ap[0]), [1, nsec],
                     list(Rbase.ap[1]), list(Rbase.ap[2])])
                eng.tensor_tensor(out=D4, in0=Lk, in1=Rk, op=OP.subtract)
                Du = Dt.bitcast(U16)[:, 0 : nsec * YB * W0]
                nc.vector.tensor_scalar(
                    out=Du, in0=Du, scalar1=0x7FFF, scalar2=None,
                    op0=OP.bitwise_and,
                )
                D3 = Dt.rearrange("p (s f) -> p s f", s=2)
                # flipped channel sum: cost[x, (sec, xb, yb, y32)] --
                # xb-major so pred's per-xb M1 weight slices are contiguous
                cost = costp.tile([XB, 512], F32, name="cost", tag="cost")
                e = ep.tile([XB, 512], FP16, name="e", tag="e")
                for sec in range(nsec):
                    for yb in range(YB):
                        for xb in range(2):
                            nc.tensor.matmul(
                                cost[0:XB,
                                     sec * 256 + xb * 128 + yb * 32 :
                                     sec * 256 + xb * 128 + yb * 32 + 32],
                                lhsT=D3[:, sec,
                                        yb * W0 + xb * XB :
                                        yb * W0 + xb * XB + XB],
                                rhs=sel,
                                start=(sec == 0 and yb == 0 and xb == 0),
                                stop=(sec == nsec - 1
                                      and yb == YB - 1 and xb == 1),
                                skip_group_check=True,
                            )
                nc.scalar.activation(out=e[:, 0 : nsec * 256],
                                     in_=cost[:, 0 : nsec * 256],
                                     func=AF.Exp, bias=bias8, scale=-1.0)
                for sec in range(nsec):
                    _emit_st_sec(h, d0 + (nsec - 1 - sec), sec, e, last_g
                                 and sec == nsec - 1)

            def _emit_st_sec(h, d, sec, e, last_sec):
                # fat accumulate over a whole [120, 256] section:
                # st layout [s(yb,xb,y32) | t(yb,xb,y32)] matches e's order
                first = not st_open[h]
                st_open[h] = True
                rhs = e[:, sec * 256 : sec * 256 + 256]
                nc.tensor.matmul(
                    st[h][0:XB, 0:256], lhsT=sid, rhs=rhs,
                    start=first, stop=False, skip_group_check=True,
                )
                nc.tensor.matmul(
                    st[h][0:XB, 256:512],
                    lhsT=tid[:, d * XB : d * XB + XB], rhs=rhs,
                    start=False, stop=last_sec, skip_group_check=True,
                )

            def emit_pred(h):
                pr = predp.tile([XB, 256], FP16, name=f"pred{h}",
                                tag=f"pred{h}")
                if cfg["divide"]:
                    nc.vector.tensor_tensor(out=pr, in0=st[h][:, 256:512],
                                            in1=st[h][:, 0:256],
                                            op=OP.divide)
                else:
                    rs = predp.tile([XB, 256], F32, name=f"rs{h}",
                                    tag=f"rs{h}")
                    nc.vector.reciprocal(out=rs, in_=st[h][:, 0:256])
                    nc.vector.tensor_tensor(out=pr, in0=st[h][:, 256:512],
                                            in1=rs, op=OP.mult)
                # pred layout [x, (xb, yb, y32)] = [x, (xb, y128)]
                pred[h] = pr

            def mid_tile():
                if cfg["fat_mid"]:
                    return outps.tile([128, 2048], F32, name="o_ps",
                                      tag="o_ps")
                return outps.tile([128, 1024], F32, name="o_ps",
                                  tag="o_ps")

            def emit_ph2_head(h, copy_fn, pool_fns, narrow=False):
                """M1 -> merged tmp [128, WP].  narrow: per-chunk tiles
                (<=512 wide); else two fat half copies from >=2-bank
                tiles."""
                lhs = [pred[h][:, xb * 128 : xb * 128 + 128]
                       for xb in range(2)]
                tmp = upsb.tile([128, WP], FP16, tag=f"tmp{h}",
                                name=f"tmp{h}")
                if narrow:
                    for c0, nw, halves in XCH_M1:
                        t_ps = pool_fns[0]()
                        for i, xb in enumerate(halves):
                            nc.tensor.matmul(
                                t_ps[:, 0:nw], lhsT=lhs[xb],
                                rhs=wxT[xb][:, c0 : c0 + nw],
                                start=(i == 0), stop=(i == len(halves) - 1),
                            )
                        copy_fn(tmp[:, c0 : c0 + nw], t_ps[:, 0:nw])
                    return tmp
                for half, (lo, hi, base, wtot) in enumerate(
                        ((0, 2, 0, 956), (2, 5, 956, WP - 956))):
                    t_ps = pool_fns[half]()
                    for c0, nw, halves in XCH_M1[lo:hi]:
                        for i, xb in enumerate(halves):
                            nc.tensor.matmul(
                                t_ps[:, c0 - base : c0 - base + nw],
                                lhsT=lhs[xb],
                                rhs=wxT[xb][:, c0 : c0 + nw],
                                start=(i == 0),
                                stop=(i == len(halves) - 1),
                            )
                    copy_fn(tmp[:, base : base + wtot], t_ps[:, 0:wtot])
                return tmp

            mid_dma_tick = [0]
            tail_dma_tick = [0]

            def emit_ph2_yc(h, tmp, yc, copy_fn, tail_tile=None,
                            last_row=False):
                """tail_tile=None: two 2-bank chunks + one full-row DMA
                (mid rows).  tail_tile: 2-bank tiles, one copy + one DMA
                per half-row (tail).  last_row: 4-way chunked DMA across
                queues to shorten the final drain."""
                ob = outsb.tile([128, WP], FP16, name="ob", tag="ob")
                wy_col = wyT[:, yc * 128 : yc * 128 + 128]
                tile_fn = tail_tile or mid_tile
                fat = cfg["fat_tail"] if tail_tile else cfg["fat_mid"]
                if tail_tile is not None and cfg["tail_chunks"] == 4:
                    chunks = tuple(XCH)
                elif fat:
                    chunks = ((0, WP),)
                else:
                    chunks = ((0, 1024), (1024, WP - 1024))
                for c0, cw in chunks:
                    o_ps = tile_fn()
                    for xc0, xnw in XCH:
                        if xc0 < c0 or xc0 >= c0 + cw:
                            continue
                        nc.tensor.matmul(
                            o_ps[:, xc0 - c0 : xc0 - c0 + xnw],
                            lhsT=wy_col, rhs=tmp[:, xc0 : xc0 + xnw],
                            start=True, stop=True,
                        )
                    copy_fn(ob[:, c0 : c0 + cw], o_ps[:, 0:cw])
                    if tail_tile is None:
                        continue
                    if last_row:
                        hw_ = cw // 2
                        lq = cfg["lr_q"]
                        for q, (d0, dw) in zip(
                                lq[0:2] if c0 == 0 else lq[2:4],
                                ((c0, hw_), (c0 + hw_, cw - hw_))):
                            ENG[q].dma_start(
                                out=out[h, yc * 128 : yc * 128 + 128,
                                        d0 : d0 + dw],
                                in_=ob[:, d0 : d0 + dw])
                        continue
                    dma_splits = ((0, 960), (960, 960)) if fat \
                        else ((0, cw),)
                    for ds, dw in dma_splits:
                        eng = ENG[TAIL_DMA[tail_dma_tick[0]
                                           % len(TAIL_DMA)]]
                        tail_dma_tick[0] += 1
                        eng.dma_start(
                            out=out[h, yc * 128 : yc * 128 + 128,
                                    c0 + ds : c0 + ds + dw],
                            in_=ob[:, c0 + ds : c0 + ds + dw])
                if tail_tile is None:
                    eng = ENG[MID_DMA[mid_dma_tick[0] % len(MID_DMA)]]
                    mid_dma_tick[0] += 1
                    eng.dma_start(
                        out=out[h, yc * 128 : yc * 128 + 128, :], in_=ob)

            # phase 1 per sample in CFG group order (P=Pool, V=DVE
            # subtract); groups are 2 disparities each (d0 = 2*g)
            GRP = [(2 * g, 2) for g in range(12)]
            ORD0 = list(cfg["order0"])
            ORD1 = list(cfg["order1"])
            for i, (g, e) in enumerate(ORD0):
                emit_ph1_group(0, *GRP[g],
                               eng=nc.gpsimd if e == "P" else None,
                               last_g=(i == len(ORD0) - 1))
            g0, e0 = ORD1[0]
            emit_ph1_group(1, *GRP[g0],
                           eng=nc.gpsimd if e0 == "P" else None)
            emit_pred(0)
            tmp0 = emit_ph2_head(0, copy_mid, (mid_tile, mid_tile))
            MR = cfg["mid_rows"]
            mid_done = 0
            for i in range(1, len(ORD1)):
                g, e = ORD1[i]
                emit_ph1_group(1, *GRP[g],
                               eng=nc.gpsimd if e == "P" else None,
                               last_g=(i == len(ORD1) - 1))
                # rows 0..MR-1 of sample 0, spread over the slots
                want = (i * MR) // (len(ORD1) - 1)
                while mid_done < want:
                    emit_ph2_yc(0, tmp0, mid_done, copy_mid)
                    mid_done += 1
            emit_pred(1)
            ph1_stack.close()  # free cost (2) + s/t (2) + out (4) banks
            tail_bufs = {2: 4, 4: 8}[cfg["tail_chunks"]] \
                if not cfg["fat_tail"] else 2
            tail_w = 2048 if cfg["fat_tail"] else \
                {2: 1024, 4: 512}[cfg["tail_chunks"]]
            with tc.tile_pool(name="pstail", bufs=tail_bufs,
                              space="PSUM") as pstail:
                def tail_tile():
                    return pstail.tile([128, tail_w], F32, name="tl",
                                       tag="tl")

                for yc in range(mid_done, 8):
                    emit_ph2_yc(0, tmp0, yc, copy_tail, tail_tile)
                tmp1 = emit_ph2_head(1, copy_tail, (tail_tile, tail_tile),
                                     narrow=(cfg["tail_chunks"] == 4))
                for yc in range(8):
                    emit_ph2_yc(1, tmp1, yc, copy_tail, tail_tile,
                                last_row=(yc >= 8 - cfg["lastrow"]))
    nc.compile()
    return nc


_NC_CACHE = [None]


def kernel(feat_l, feat_r, img_h, img_w):
    feat_l = np.asarray(feat_l, dtype=np.float32)
    feat_r = np.asarray(feat_r, dtype=np.float32)
    assert int(img_h) == HI and int(img_w) == WI
    assert feat_l.shape == (B, C, H0, W0)

    if _NC_CACHE[0] is None:
        _NC_CACHE[0] = _build()
    nc = _NC_CACHE[0]

    consts = _host_consts()
    in_maps = []
    for c in range(NCORES):
        fl = _pack_feat(feat_l[SPC * c : SPC * c + SPC].astype(np.float16))
        fr = _pack_feat_padded(
            feat_r[SPC * c : SPC * c + SPC].astype(np.float16))
        in_maps.append({"lf": fl, "rf": fr, **consts})

    res = run_bass_kernel_spmd(nc, in_maps, core_ids=list(range(NCORES)),
                               trace=_TRACE[0])
    outs = [res.results[i]["out"].astype(np.float32) for i in range(NCORES)]
    full = np.concatenate(outs, axis=0).reshape(B, 1, HI, WI)
    kernel._last_exec_ns = res.exec_time_ns
    return full
